# revision 15
# baseline (speedup 1.0000x reference)
"""Binarized 3x3 conv + bias + ReLU + eval-mode BatchNorm, Trainium2 Bass kernel.

Problem: x[16,64,256,256] f32, w[64,64,3,3], per-channel b/gamma/beta/mean/var.
  y = BN(relu(conv(sign(x), sign(w)) + b))  (eval-mode BN = per-channel affine)

Strategy (8 NeuronCores, data-parallel over batch):
  - 2 images per core; image A on SBUF partitions 0-63 (channels), image B on 64-127.
  - Binarize on-chip on ScalarE: xb = Sign(x) in {-1,+1} bf16; spatial padding
    is 0.0 (contributes nothing), so psum = conv(sign(w), sign(x)) exactly.
  - 3x3 conv = 9 accumulating matmuls per PSUM tile (K=Cin=64, M=Cout=64),
    using 64x64 PE array tiling: 4 quadrants = (imgA,imgB) x (left,right 128-col
    half) run concurrently -> full 128x128 array utilization.
  - Post: ScalarE relu(psum + b) then VectorE y = t*inv + c (per-partition
    vectors), into 8-row staging tiles -> 2 MB output DMAs.
  - ALL DMAs are 128-partition HWDGE.  Input rows ride the ACT ring
    (nc.scalar; dispatch never has to wait), outputs ride the otherwise-idle
    SP ring (nc.sync; ring-credit waits block nothing).  No SWDGE/gpsimd.
  - FIFO placement discipline: every xb-prep op is emitted right after an
    instruction whose own wait already implies the prep's dependency, so no
    engine queue ever head-of-line blocks:
      * binarize chunks for block b+1 go after block b's supertile-0/1
        activations (xb tile free is implied by those acts' psum waits);
      * halo rows for block b+1 are binarized from block b's xin tail (no
        copy, no extra HBM read);
      * pad memsets for block b+2 go after block b's final BN-affine on DVE.
  - PSUM bank B holds image-swapped halves (PE quadrant packing); its rows are
    written to the *other* image's row range in DRAM and the swap is undone on
    the host during the gather (free - not on the device critical path).
"""

import numpy as np
import ml_dtypes

import concourse.bass as bass  # noqa: F401  (AP types ride along)
import concourse.mybir as mybir
import concourse.tile as tile
from concourse import bacc
from concourse.bass_utils import run_bass_kernel_spmd

N_CORES = 8
IMGS_PER_CORE = 2
C = 64
H = 256
W = 256
RB = 32              # output rows per block
NBLK = H // RB       # 8
ROWS_IN = RB + 2     # xb rows: halo row above + 32 outputs + halo row below
WP = W + 4           # padded row width in xb; data at col offset 2
BN_EPS = 1e-5
DT = mybir.dt
SIGN = mybir.ActivationFunctionType.Sign

_PROGRAM = None


def _build():
    nc = bacc.Bacc(
        "TRN2",
        target_bir_lowering=False,
        debug=False,
        enable_asserts=False,
    )
    x = nc.dram_tensor("x", [IMGS_PER_CORE, C, H, W], DT.float32, kind="ExternalInput")
    wT = nc.dram_tensor("wT", [128, 9 * 64], DT.bfloat16, kind="ExternalInput")
    bvec = nc.dram_tensor("bvec", [128, 1], DT.float32, kind="ExternalInput")
    ivec = nc.dram_tensor("ivec", [128, 1], DT.float32, kind="ExternalInput")
    cvec = nc.dram_tensor("cvec", [128, 1], DT.float32, kind="ExternalInput")
    y = nc.dram_tensor("y", [IMGS_PER_CORE, C, H, W], DT.float32, kind="ExternalOutput")

    x_flat = x.ap().rearrange("n c h w -> (n c) (h w)")   # [128, 65536] flat
    y_m = y.ap().rearrange("n c h w -> (n c) (h w)")      # [128, 65536] flat

    with tile.TileContext(nc) as tc:
        with (
            tc.tile_pool(name="consts", bufs=1) as cpool,
            tc.tile_pool(name="xin", bufs=3) as xpool,
            tc.tile_pool(name="xbp", bufs=2) as xbpool,
            tc.tile_pool(name="tsb", bufs=4) as tpool,
            tc.tile_pool(name="yout", bufs=5) as ypool,
            tc.tile_pool(name="psum", bufs=2, space="PSUM") as ppool,
        ):
            wt = cpool.tile([128, 9 * 64], DT.bfloat16, tag="wt")
            bv = cpool.tile([128, 1], DT.float32, tag="bv")
            iv = cpool.tile([128, 1], DT.float32, tag="iv")
            cv = cpool.tile([128, 1], DT.float32, tag="cv")

            xins = {}   # blk -> xin view [128, rows, W]
            xbs = {}    # blk -> xb view [128, 34, WP]

            def load_consts():
                nc.sync.dma_start(wt[:], wT.ap())
                nc.sync.dma_start(bv[:], bvec.ap())
                nc.sync.dma_start(iv[:], ivec.ap())
                nc.sync.dma_start(cv[:], cvec.ap())

            def alloc_xb(blk):
                """Allocate block's xb tile and memset its pads.  Emitted at a
                point where the tile's recycling wait is already satisfied."""
                xb = xbpool.tile([128, ROWS_IN * WP], DT.bfloat16, tag="xb")
                xb_v = xb[:].rearrange("p (r c) -> p r c", c=WP)
                xbs[blk] = xb_v
                nc.vector.memset(xb_v[:, :, 0:2], 0.0)
                nc.vector.memset(xb_v[:, :, 2 + W : WP], 0.0)
                if blk == 0:
                    nc.vector.memset(xb_v[:, 0:1, :], 0.0)
                if blk == NBLK - 1:
                    nc.vector.memset(xb_v[:, 33:34, :], 0.0)
                return xb_v

            def load_dma(blk):
                """Input DMA only (ACT ring).  xb row k <-> x row blk*RB-1+k."""
                r0 = blk * RB
                nrows = 33 if blk == 0 else (31 if blk == NBLK - 1 else 32)
                xin = xpool.tile([128, 33 * W], DT.float32, tag="xin")
                xin_v = xin[:].rearrange("p (r c) -> p r c", c=W)
                xins[blk] = xin_v
                if blk == 0:
                    # x rows 0..32 in two chunks so binarize starts early
                    nc.scalar.dma_start(xin[:, 0 : 17 * W], x_flat[:, 0 : 17 * W])
                    nc.scalar.dma_start(
                        xin[:, 17 * W : 33 * W], x_flat[:, 17 * W : 33 * W]
                    )
                else:
                    # x rows r0+1 .. r0+nrows
                    nc.scalar.dma_start(
                        xin[:, 0 : nrows * W],
                        x_flat[:, (r0 + 1) * W : (r0 + 1 + nrows) * W],
                    )

            def binz(dst, rows_dst, src, rows_src):
                nc.scalar.activation(
                    dst[:, rows_dst[0] : rows_dst[1], 2 : 2 + W],
                    src[:, rows_src[0] : rows_src[1], :],
                    SIGN,
                )

            def emit_binz(blk, part):
                """Binarize chunks for a block on ScalarE.  part 0: halo rows
                (from previous block's xin tail) + first 2 of 4 body chunks;
                part 1: last 2 body chunks."""
                xb_v = xbs[blk]
                if blk == 0:
                    off = 1   # xb row k <- xin row k-1 (xin holds x rows 0..32)
                    bounds = ((1, 9), (9, 17), (17, 25), (25, 34))
                else:
                    off = 2   # xb row k <- xin row k-2 (xin holds x rows r0+1..)
                    n_new = 31 if blk == NBLK - 1 else 32
                    bounds = ((2, 10), (10, 18), (18, 26), (26, 2 + n_new))
                if part == 0:
                    if blk > 0:
                        hs = 31 if blk == 1 else 30
                        binz(xb_v, (0, 2), xins[blk - 1], (hs, hs + 2))
                    sel = bounds[0:2]
                else:
                    sel = bounds[2:4]
                for a, b in sel:
                    binz(xb_v, (a, b), xins[blk], (a - off, b - off))

            def compute_block(blk):
                """Matmuls + post-ops + output DMAs for a loaded block; also
                emits next block's binarize and block b+2's xb setup at the
                FIFO-safe points."""
                xb_v = xbs[blk]
                r0 = blk * RB
                hb = RB // 2  # rows per half-block (16)
                last = blk == NBLK - 1
                # PSUM bank T = [imgA-top | imgB-top] (partition = n*64+c);
                # bank B = [imgB-bot | imgA-bot] (image-reversed; undone on host).
                # Quadrants: A-T=(0,0)  B-T=(64,64)  B-B=(64,0)  A-B=(0,64)
                ytiles = {}
                for it2 in range(hb // 4):          # 4 output rows per super-tile
                    ps_t = ppool.tile([128, 1024], DT.float32, tag="pst")
                    ps_b = ppool.tile([128, 1024], DT.float32, tag="psb")
                    for sub in range(2):            # 2 rows per matmul set
                        it = 2 * it2 + sub
                        c0 = sub * 512
                        for t in range(9):
                            dy, dx = divmod(t, 3)
                            first, lastt = (t == 0), (t == 8)
                            rt = 2 * it + dy              # top-half rows
                            rb_ = hb + 2 * it + dy        # bottom-half rows
                            cs = 1 + dx
                            quads = (
                                (ps_t, 0, 0, rt),      # A-top -> psT[0:64]
                                (ps_t, 64, 64, rt),    # B-top -> psT[64:128]
                                (ps_b, 64, 0, rb_),    # B-bot -> psB[0:64]
                                (ps_b, 0, 64, rb_),    # A-bot -> psB[64:128]
                            )
                            for ps, xp0, op0_, rlo in quads:
                                wslc = wt[xp0 : xp0 + 64, t * 64 : (t + 1) * 64]
                                rhs = xb_v[xp0 : xp0 + 64, rlo : rlo + 2, cs : cs + W]
                                nc.tensor.matmul(
                                    ps[op0_ : op0_ + 64, c0 : c0 + 512],
                                    wslc,
                                    rhs,
                                    start=first,
                                    stop=lastt,
                                )
                    # drain both banks: relu+bias (ACT), BN affine (DVE) into
                    # 8-row staging tiles; 2 MB output DMA (SP ring) per pair
                    # of super-tiles (1 MB per super-tile for the last block).
                    half = it2 & 1
                    for bank, (ps, roff) in enumerate(
                        ((ps_t, (it2 // 2) * 8), (ps_b, hb + (it2 // 2) * 8))
                    ):
                        tsb = tpool.tile([128, 1024], DT.float32, tag="tsb")
                        # relu(ps + b): bank B on ScalarE, bank T on VectorE,
                        # so the per-super-tile drain pace on EACH engine stays
                        # below the PE's per-super-tile matmul time (else the
                        # PE hits the PSUM-recycle wall and HAM-cools)
                        if bank == 1:
                            nc.scalar.activation(
                                tsb[:],
                                ps[:],
                                mybir.ActivationFunctionType.Relu,
                                bias=bv[:],
                            )
                        else:
                            nc.vector.tensor_scalar(
                                tsb[:],
                                ps[:],
                                bv[:],
                                0.0,
                                op0=mybir.AluOpType.add,
                                op1=mybir.AluOpType.max,
                            )
                        if half == 0:
                            ytiles[bank] = ypool.tile(
                                [128, 2048], DT.float32, tag="yst", name="yst"
                            )
                        yst = ytiles[bank]
                        nc.vector.tensor_scalar(
                            yst[:, half * 1024 : half * 1024 + 1024],
                            tsb[:],
                            iv[:],
                            cv[:],
                            op0=mybir.AluOpType.mult,
                            op1=mybir.AluOpType.add,
                        )
                        if last:
                            # fine-grained tail flush: 1 MB per super-tile
                            nc.sync.dma_start(
                                y_m[
                                    :,
                                    (r0 + roff + half * 4) * W
                                    : (r0 + roff + half * 4 + 4) * W,
                                ],
                                yst[:, half * 1024 : half * 1024 + 1024],
                            )
                        elif half == 1:
                            nc.sync.dma_start(
                                y_m[:, (r0 + roff) * W : (r0 + roff + 8) * W],
                                yst[:],
                            )
                    # next block's binarize at the FIFO-safe points
                    if blk + 1 < NBLK and it2 in (0, 1):
                        emit_binz(blk + 1, it2)
                # block b+2's xb tile + pad memsets (DVE wait already implied)
                if blk + 2 < NBLK:
                    alloc_xb(blk + 2)

            # ---- pipeline ----
            load_dma(0)
            alloc_xb(0)
            emit_binz(0, 0)
            load_dma(1)
            load_consts()
            emit_binz(0, 1)
            alloc_xb(1)
            emit_binz(1, 0)
            emit_binz(1, 1)
            load_dma(2)
            for blk in range(NBLK):
                compute_block(blk)
                if blk + 3 < NBLK:
                    load_dma(blk + 3)
    nc.compile()
    return nc


def _get_program():
    global _PROGRAM
    if _PROGRAM is None:
        _PROGRAM = _build()
    return _PROGRAM


def _prep_params(w, b, gamma, beta, running_mean, running_var):
    wb = np.where(w >= 0, 1.0, -1.0).astype(np.float32)          # [co, ci, ky, kx]
    wt = np.ascontiguousarray(wb.transpose(1, 2, 3, 0))          # [ci, ky, kx, co]
    wt = wt.reshape(C, 9 * C).astype(ml_dtypes.bfloat16)
    wt2 = np.ascontiguousarray(np.concatenate([wt, wt], axis=0))  # [128, 576]
    inv = (gamma.astype(np.float32) / np.sqrt(running_var.astype(np.float32) + BN_EPS)).astype(np.float32)
    cc = (beta.astype(np.float32) - running_mean.astype(np.float32) * inv).astype(np.float32)
    bp = b.astype(np.float32)

    def rep(v):
        return np.ascontiguousarray(np.tile(v.astype(np.float32), 2).reshape(128, 1))

    return wt2, rep(bp), rep(inv), rep(cc)


def _unswizzle(yd):
    """Undo the on-device image swap of the bottom 16-row half of each
    32-row block (PSUM bank B holds image-reversed partitions)."""
    v = yd.reshape(IMGS_PER_CORE, C, NBLK, 2, RB // 2, W)
    out = np.empty_like(v)
    out[:, :, :, 0] = v[:, :, :, 0]
    out[:, :, :, 1] = v[::-1, :, :, 1]
    return out.reshape(IMGS_PER_CORE, C, H, W)


def run(x, w, b, gamma, beta, running_mean, running_var, trace=False):
    nc = _get_program()
    wt2, bp, inv, cc = _prep_params(w, b, gamma, beta, running_mean, running_var)
    x = np.asarray(x, dtype=np.float32)
    in_maps = []
    for i in range(N_CORES):
        in_maps.append(
            {
                "x": np.ascontiguousarray(x[IMGS_PER_CORE * i : IMGS_PER_CORE * (i + 1)]),
                "wT": wt2,
                "bvec": bp,
                "ivec": inv,
                "cvec": cc,
            }
        )
    res = run_bass_kernel_spmd(nc, in_maps, list(range(N_CORES)), trace=trace)
    y = np.concatenate(
        [_unswizzle(res.results[i]["y"]) for i in range(N_CORES)], axis=0
    )
    return y, res


def kernel(x, w, b, gamma, beta, running_mean, running_var):
    y, _ = run(x, w, b, gamma, beta, running_mean, running_var)
    return y


# revision 16
# speedup vs baseline: 1.1454x; 1.1454x over previous
"""Binarized 3x3 conv + bias + ReLU + eval-mode BatchNorm, Trainium2 Bass kernel.

Problem: x[16,64,256,256] f32, w[64,64,3,3], per-channel b/gamma/beta/mean/var.
  y = BN(relu(conv(sign(x), sign(w)) + b))  (eval-mode BN = per-channel affine)

Strategy (8 NeuronCores, data-parallel over batch):
  - 2 images per core; image A on SBUF partitions 0-63 (channels), image B on 64-127.
  - Binarize on-chip on ScalarE: xb = Sign(x) in {-1,+1} bf16; spatial padding
    is 0.0 (contributes nothing), so psum = conv(sign(w), sign(x)) exactly.
  - 3x3 conv = 9 accumulating matmuls per PSUM tile (K=Cin=64, M=Cout=64),
    using 64x64 PE array tiling: 4 quadrants = (imgA,imgB) x (left,right 128-col
    half) run concurrently -> full 128x128 array utilization.
  - Post: ScalarE relu(psum + b) then VectorE y = t*inv + c (per-partition
    vectors), into 8-row staging tiles -> 2 MB output DMAs.
  - ALL DMAs are 128-partition HWDGE.  Input rows ride the ACT ring
    (nc.scalar; dispatch never has to wait), outputs ride the otherwise-idle
    SP ring (nc.sync; ring-credit waits block nothing).  No SWDGE/gpsimd.
  - FIFO placement discipline: every xb-prep op is emitted right after an
    instruction whose own wait already implies the prep's dependency, so no
    engine queue ever head-of-line blocks:
      * binarize chunks for block b+1 go after block b's supertile-0/1
        activations (xb tile free is implied by those acts' psum waits);
      * halo rows for block b+1 are binarized from block b's xin tail (no
        copy, no extra HBM read);
      * pad memsets for block b+2 go after block b's final BN-affine on DVE.
  - PSUM bank B holds image-swapped halves (PE quadrant packing); its rows are
    written to the *other* image's row range in DRAM and the swap is undone on
    the host during the gather (free - not on the device critical path).
"""

import numpy as np
import ml_dtypes

import concourse.bass as bass  # noqa: F401  (AP types ride along)
import concourse.mybir as mybir
import concourse.tile as tile
from concourse import bacc
from concourse.bass_utils import run_bass_kernel_spmd

N_CORES = 8
IMGS_PER_CORE = 2
C = 64
H = 256
W = 256
RB = 32              # output rows per block
NBLK = H // RB       # 8
ROWS_IN = RB + 2     # xb rows: halo row above + 32 outputs + halo row below
WP = W + 4           # padded row width in xb; data at col offset 2
BN_EPS = 1e-5
DT = mybir.dt
SIGN = mybir.ActivationFunctionType.Sign

_PROGRAM = None


def _build():
    nc = bacc.Bacc(
        "TRN2",
        target_bir_lowering=False,
        debug=False,
        enable_asserts=False,
    )
    x = nc.dram_tensor("x", [IMGS_PER_CORE, C, H, W], DT.float32, kind="ExternalInput")
    wT = nc.dram_tensor("wT", [128, 9 * 64], DT.bfloat16, kind="ExternalInput")
    bvec = nc.dram_tensor("bvec", [128, 1], DT.float32, kind="ExternalInput")
    ivec = nc.dram_tensor("ivec", [128, 1], DT.float32, kind="ExternalInput")
    cvec = nc.dram_tensor("cvec", [128, 1], DT.float32, kind="ExternalInput")
    y = nc.dram_tensor("y", [IMGS_PER_CORE, C, H, W], DT.float32, kind="ExternalOutput")

    x_flat = x.ap().rearrange("n c h w -> (n c) (h w)")   # [128, 65536] flat
    y_m = y.ap().rearrange("n c h w -> (n c) (h w)")      # [128, 65536] flat

    with tile.TileContext(nc) as tc:
        with (
            tc.tile_pool(name="consts", bufs=1) as cpool,
            tc.tile_pool(name="xin", bufs=3) as xpool,
            tc.tile_pool(name="xbp", bufs=2) as xbpool,
            tc.tile_pool(name="tsb", bufs=6) as tpool,
            tc.tile_pool(name="yout", bufs=5) as ypool,
            tc.tile_pool(name="psum", bufs=2, space="PSUM") as ppool,
        ):
            wt = cpool.tile([128, 9 * 64], DT.bfloat16, tag="wt")
            bv = cpool.tile([128, 1], DT.float32, tag="bv")
            iv = cpool.tile([128, 1], DT.float32, tag="iv")
            cv = cpool.tile([128, 1], DT.float32, tag="cv")

            xins = {}   # blk -> xin view [128, rows, W]
            xbs = {}    # blk -> xb view [128, 34, WP]

            def load_consts():
                nc.sync.dma_start(wt[:], wT.ap())
                nc.sync.dma_start(bv[:], bvec.ap())
                nc.sync.dma_start(iv[:], ivec.ap())
                nc.sync.dma_start(cv[:], cvec.ap())

            def alloc_xb(blk):
                """Allocate block's xb tile and memset its pads.  Emitted at a
                point where the tile's recycling wait is already satisfied."""
                xb = xbpool.tile([128, ROWS_IN * WP], DT.bfloat16, tag="xb")
                xb_v = xb[:].rearrange("p (r c) -> p r c", c=WP)
                xbs[blk] = xb_v
                nc.vector.memset(xb_v[:, :, 0:2], 0.0)
                nc.vector.memset(xb_v[:, :, 2 + W : WP], 0.0)
                if blk == 0:
                    nc.vector.memset(xb_v[:, 0:1, :], 0.0)
                if blk == NBLK - 1:
                    nc.vector.memset(xb_v[:, 33:34, :], 0.0)
                return xb_v

            def load_dma(blk):
                """Input DMA only (ACT ring).  xb row k <-> x row blk*RB-1+k."""
                r0 = blk * RB
                nrows = 33 if blk == 0 else (31 if blk == NBLK - 1 else 32)
                xin = xpool.tile([128, 33 * W], DT.float32, tag="xin")
                xin_v = xin[:].rearrange("p (r c) -> p r c", c=W)
                xins[blk] = xin_v
                if blk == 0:
                    # x rows 0..32 in two chunks so binarize starts early
                    nc.scalar.dma_start(xin[:, 0 : 17 * W], x_flat[:, 0 : 17 * W])
                    nc.scalar.dma_start(
                        xin[:, 17 * W : 33 * W], x_flat[:, 17 * W : 33 * W]
                    )
                else:
                    # x rows r0+1 .. r0+nrows
                    nc.scalar.dma_start(
                        xin[:, 0 : nrows * W],
                        x_flat[:, (r0 + 1) * W : (r0 + 1 + nrows) * W],
                    )

            def binz(dst, rows_dst, src, rows_src):
                nc.scalar.activation(
                    dst[:, rows_dst[0] : rows_dst[1], 2 : 2 + W],
                    src[:, rows_src[0] : rows_src[1], :],
                    SIGN,
                )

            def emit_binz(blk, part):
                """Binarize chunks for a block on ScalarE.  part 0: halo rows
                (from previous block's xin tail) + first 2 of 4 body chunks;
                part 1: last 2 body chunks."""
                xb_v = xbs[blk]
                if blk == 0:
                    off = 1   # xb row k <- xin row k-1 (xin holds x rows 0..32)
                    bounds = ((1, 9), (9, 17), (17, 25), (25, 34))
                else:
                    off = 2   # xb row k <- xin row k-2 (xin holds x rows r0+1..)
                    n_new = 31 if blk == NBLK - 1 else 32
                    bounds = ((2, 10), (10, 18), (18, 26), (26, 2 + n_new))
                if part == 0:
                    if blk > 0:
                        hs = 31 if blk == 1 else 30
                        binz(xb_v, (0, 2), xins[blk - 1], (hs, hs + 2))
                    sel = bounds[0:2]
                else:
                    sel = bounds[2:4]
                for a, b in sel:
                    binz(xb_v, (a, b), xins[blk], (a - off, b - off))

            def compute_block(blk):
                """Matmuls + post-ops + output DMAs for a loaded block; also
                emits next block's binarize and block b+2's xb setup at the
                FIFO-safe points."""
                xb_v = xbs[blk]
                r0 = blk * RB
                hb = RB // 2  # rows per half-block (16)
                last = blk == NBLK - 1
                # PSUM bank T = [imgA-top | imgB-top] (partition = n*64+c);
                # bank B = [imgB-bot | imgA-bot] (image-reversed; undone on host).
                # Quadrants: A-T=(0,0)  B-T=(64,64)  B-B=(64,0)  A-B=(0,64)
                ytiles = {}
                for it2 in range(hb // 4):          # 4 output rows per super-tile
                    ps_t = ppool.tile([128, 1024], DT.float32, tag="pst")
                    ps_b = ppool.tile([128, 1024], DT.float32, tag="psb")
                    for sub in range(2):            # 2 rows per matmul set
                        it = 2 * it2 + sub
                        c0 = sub * 512
                        for t in range(9):
                            dy, dx = divmod(t, 3)
                            first, lastt = (t == 0), (t == 8)
                            rt = 2 * it + dy              # top-half rows
                            rb_ = hb + 2 * it + dy        # bottom-half rows
                            cs = 1 + dx
                            quads = (
                                (ps_t, 0, 0, rt),      # A-top -> psT[0:64]
                                (ps_t, 64, 64, rt),    # B-top -> psT[64:128]
                                (ps_b, 64, 0, rb_),    # B-bot -> psB[0:64]
                                (ps_b, 0, 64, rb_),    # A-bot -> psB[64:128]
                            )
                            for ps, xp0, op0_, rlo in quads:
                                wslc = wt[xp0 : xp0 + 64, t * 64 : (t + 1) * 64]
                                rhs = xb_v[xp0 : xp0 + 64, rlo : rlo + 2, cs : cs + W]
                                nc.tensor.matmul(
                                    ps[op0_ : op0_ + 64, c0 : c0 + 512],
                                    wslc,
                                    rhs,
                                    start=first,
                                    stop=lastt,
                                )
                    # drain both banks: relu+bias (ACT), BN affine (DVE) into
                    # 8-row staging tiles; 2 MB output DMA (SP ring) per pair
                    # of super-tiles (1 MB per super-tile for the last block).
                    half = it2 & 1
                    for bank, (ps, roff) in enumerate(
                        ((ps_t, (it2 // 2) * 8), (ps_b, hb + (it2 // 2) * 8))
                    ):
                        tsb = tpool.tile([128, 1024], DT.float32, tag="tsb")
                        # relu(ps + b): split across ScalarE and VectorE so
                        # neither engine paces slower than the PE
                        if (it2, bank) in ((0, 1), (2, 1), (3, 0)):
                            nc.scalar.activation(
                                tsb[:],
                                ps[:],
                                mybir.ActivationFunctionType.Relu,
                                bias=bv[:],
                            )
                        else:
                            nc.vector.tensor_scalar(
                                tsb[:],
                                ps[:],
                                bv[:],
                                0.0,
                                op0=mybir.AluOpType.add,
                                op1=mybir.AluOpType.max,
                            )
                        if half == 0:
                            ytiles[bank] = ypool.tile(
                                [128, 2048], DT.float32, tag="yst", name="yst"
                            )
                        yst = ytiles[bank]
                        nc.vector.tensor_scalar(
                            yst[:, half * 1024 : half * 1024 + 1024],
                            tsb[:],
                            iv[:],
                            cv[:],
                            op0=mybir.AluOpType.mult,
                            op1=mybir.AluOpType.add,
                        )
                        if last:
                            # fine-grained tail flush: 1 MB per super-tile
                            nc.sync.dma_start(
                                y_m[
                                    :,
                                    (r0 + roff + half * 4) * W
                                    : (r0 + roff + half * 4 + 4) * W,
                                ],
                                yst[:, half * 1024 : half * 1024 + 1024],
                            )
                        elif half == 1:
                            nc.sync.dma_start(
                                y_m[:, (r0 + roff) * W : (r0 + roff + 8) * W],
                                yst[:],
                            )
                    # next block's binarize at the FIFO-safe points
                    if blk + 1 < NBLK and it2 in (0, 1):
                        emit_binz(blk + 1, it2)
                # block b+2's xb tile + pad memsets (DVE wait already implied)
                if blk + 2 < NBLK:
                    alloc_xb(blk + 2)

            # ---- pipeline ----
            load_dma(0)
            alloc_xb(0)
            emit_binz(0, 0)
            load_dma(1)
            load_consts()
            emit_binz(0, 1)
            alloc_xb(1)
            emit_binz(1, 0)
            emit_binz(1, 1)
            load_dma(2)
            for blk in range(NBLK):
                compute_block(blk)
                if blk + 3 < NBLK:
                    load_dma(blk + 3)
    nc.compile()
    return nc


def _get_program():
    global _PROGRAM
    if _PROGRAM is None:
        _PROGRAM = _build()
    return _PROGRAM


def _prep_params(w, b, gamma, beta, running_mean, running_var):
    wb = np.where(w >= 0, 1.0, -1.0).astype(np.float32)          # [co, ci, ky, kx]
    wt = np.ascontiguousarray(wb.transpose(1, 2, 3, 0))          # [ci, ky, kx, co]
    wt = wt.reshape(C, 9 * C).astype(ml_dtypes.bfloat16)
    wt2 = np.ascontiguousarray(np.concatenate([wt, wt], axis=0))  # [128, 576]
    inv = (gamma.astype(np.float32) / np.sqrt(running_var.astype(np.float32) + BN_EPS)).astype(np.float32)
    cc = (beta.astype(np.float32) - running_mean.astype(np.float32) * inv).astype(np.float32)
    bp = b.astype(np.float32)

    def rep(v):
        return np.ascontiguousarray(np.tile(v.astype(np.float32), 2).reshape(128, 1))

    return wt2, rep(bp), rep(inv), rep(cc)


def _unswizzle(yd):
    """Undo the on-device image swap of the bottom 16-row half of each
    32-row block (PSUM bank B holds image-reversed partitions)."""
    v = yd.reshape(IMGS_PER_CORE, C, NBLK, 2, RB // 2, W)
    out = np.empty_like(v)
    out[:, :, :, 0] = v[:, :, :, 0]
    out[:, :, :, 1] = v[::-1, :, :, 1]
    return out.reshape(IMGS_PER_CORE, C, H, W)


def run(x, w, b, gamma, beta, running_mean, running_var, trace=False):
    nc = _get_program()
    wt2, bp, inv, cc = _prep_params(w, b, gamma, beta, running_mean, running_var)
    x = np.asarray(x, dtype=np.float32)
    in_maps = []
    for i in range(N_CORES):
        in_maps.append(
            {
                "x": np.ascontiguousarray(x[IMGS_PER_CORE * i : IMGS_PER_CORE * (i + 1)]),
                "wT": wt2,
                "bvec": bp,
                "ivec": inv,
                "cvec": cc,
            }
        )
    res = run_bass_kernel_spmd(nc, in_maps, list(range(N_CORES)), trace=trace)
    y = np.concatenate(
        [_unswizzle(res.results[i]["y"]) for i in range(N_CORES)], axis=0
    )
    return y, res


def kernel(x, w, b, gamma, beta, running_mean, running_var):
    y, _ = run(x, w, b, gamma, beta, running_mean, running_var)
    return y
